# revision 1
# baseline (speedup 1.0000x reference)
"""CRF loss (forward-algorithm log-partition minus gold-path score) on 8 TRN2
NeuronCores.

Sharding: data-parallel over batch. B=128 -> 16 sequences per core; the small
(L,L) transition params are replicated. Each core returns a scalar partial sum
of (den[b] - num[b]) over its 16 lanes; the host adds the analytic kappa
offset and divides by B (the "all-reduce" of the mean).

Device algorithm (per core):
  Denominator: forward scan in exp space,
      e_{t+1}[j, b] = (sum_i expT[i, j] * e_t[i, b]) * P_t[j, b]
  with expT = exp(trans - kappa) in bf16 (stationary matmul weights, labels
  on partitions -> no per-step transpose) and P_t = exp(pred[t]) in
  [label, batch] layout (PE-transposed per 128-row chunk). The per-step
  critical path is one bf16 matmul (16-column rhs) + one DVE multiply.
  Every 128 steps, an exact per-lane renormalization folds 1/colsum into the
  NEXT chunk's first P slice (linearity makes deferred scaling exact) and
  tracks -ln(recip) in an offset row - fully off the critical path. bf16
  covers the full fp32 exponent range, so no over/underflow management is
  needed beyond kappa.
  den[b] = offset[b] + ln(sum_j e_T[j,b] * exp(end[j])) + (T-1)*kappa.

  Numerator (the benchmark's mask is all-ones):
    emission sum: per 128-row chunk (rows = (t, b)), one fused DVE
      scalar_tensor_tensor: (iota == tgt_row) * pred_chunk accumulated along
      the free axis.
    transition sum: pair-count matrix C[i,j] = #(t: tgt[t]=i, tgt[t+1]=j)
      accumulated across chunks as PSUM matmuls of bf16 onehot pairs, then one
      fused multiply-reduce against the raw fp32 transition table.
    start/end: tiny onehot gathers on 16 partitions.
"""

import numpy as np
from contextlib import ExitStack

import concourse.bass as bass
import concourse.bacc as bacc
import concourse.tile as tile
from concourse import mybir
from concourse.bass_utils import run_bass_kernel_spmd

T, B, L = 1024, 128, 128
NCORES = 8
BLOC = B // NCORES          # 16 batch lanes per core
ROWS = T * BLOC             # 16384 (t, b) rows per core
NCHUNK = ROWS // 128        # 128 chunks of 128 rows (8 time steps x 16 lanes)
TPC = 128 // BLOC           # 8 time steps per chunk
KAPPA = 5.9                 # mean per-step log growth; folded into expT
F32 = mybir.dt.float32
BF16 = mybir.dt.bfloat16
AX = mybir.AxisListType
OP = mybir.AluOpType
AF = mybir.ActivationFunctionType

RENORM_EVERY = 16           # renorm colsum every 16 chunks (128 steps)
N_RENORM = NCHUNK // RENORM_EVERY - 1   # 7: last window needs no renorm


def _build_program():
    nc = bacc.Bacc("TRN2", target_bir_lowering=False, debug=False,
                   num_devices=NCORES)

    pred_d = nc.dram_tensor("pred", [ROWS, L], F32, kind="ExternalInput")
    tgtf_d = nc.dram_tensor("tgtf", [128, NCHUNK], F32, kind="ExternalInput")
    tgtn_d = nc.dram_tensor("tgtn", [128, NCHUNK], F32, kind="ExternalInput")
    trans_d = nc.dram_tensor("transm", [L, L], F32, kind="ExternalInput")
    startc_d = nc.dram_tensor("startc", [L, 1], F32, kind="ExternalInput")
    endc_d = nc.dram_tensor("endc", [L, 1], F32, kind="ExternalInput")
    startr_d = nc.dram_tensor("startr", [1, L], F32, kind="ExternalInput")
    endr_d = nc.dram_tensor("endr", [1, L], F32, kind="ExternalInput")
    t0_d = nc.dram_tensor("t0c", [BLOC, 1], F32, kind="ExternalInput")
    tlast_d = nc.dram_tensor("tlastc", [BLOC, 1], F32, kind="ExternalInput")
    iota_d = nc.dram_tensor("iotar", [L, L], F32, kind="ExternalInput")
    ident_d = nc.dram_tensor("ident", [L, L], F32, kind="ExternalInput")
    ones_d = nc.dram_tensor("onesc", [L, 1], F32, kind="ExternalInput")
    out_d = nc.dram_tensor("out", [1, 1], F32, kind="ExternalOutput")

    with tile.TileContext(nc) as tc, ExitStack() as ctx:
        const = ctx.enter_context(tc.tile_pool(name="const", bufs=1))
        natp = ctx.enter_context(tc.tile_pool(name="nat", bufs=3))
        nbp = ctx.enter_context(tc.tile_pool(name="natb", bufs=4))
        pexp = ctx.enter_context(tc.tile_pool(name="pexp", bufs=4))
        scrp = ctx.enter_context(tc.tile_pool(name="scr", bufs=2))
        ohp = ctx.enter_context(tc.tile_pool(name="oh", bufs=3))
        ep = ctx.enter_context(tc.tile_pool(name="e", bufs=4))
        smallp = ctx.enter_context(tc.tile_pool(name="small", bufs=2))
        offp = ctx.enter_context(tc.tile_pool(name="offp", bufs=2))
        rbcp = ctx.enter_context(tc.tile_pool(name="rbcp", bufs=2))
        pscp = ctx.enter_context(tc.tile_pool(name="psc", bufs=2))
        zp = ctx.enter_context(tc.tile_pool(name="z", bufs=3, space="PSUM"))
        ptp = ctx.enter_context(tc.tile_pool(name="pt", bufs=2, space="PSUM"))
        cp = ctx.enter_context(tc.tile_pool(name="cmat", bufs=1, space="PSUM"))
        rp = ctx.enter_context(tc.tile_pool(name="rsm", bufs=1, space="PSUM"))

        # ---- one-time constants into SBUF ----
        def load_const(name, shape, dram):
            t = const.tile(shape, F32, tag=name)
            nc.sync.dma_start(t[:], dram.ap())
            return t

        trans_s = load_const("trans_s", [L, L], trans_d)
        iota_s = load_const("iota_s", [L, L], iota_d)
        ident_s = load_const("ident_s", [L, L], ident_d)
        ones_s = load_const("ones_s", [L, 1], ones_d)
        startc_s = load_const("startc_s", [L, 1], startc_d)
        endc_s = load_const("endc_s", [L, 1], endc_d)
        startr_s = load_const("startr_s", [1, L], startr_d)
        endr_s = load_const("endr_s", [1, L], endr_d)
        tgtf_s = load_const("tgtf_s", [128, NCHUNK], tgtf_d)
        tgtn_s = load_const("tgtn_s", [128, NCHUNK], tgtn_d)
        t0_s = load_const("t0_s", [BLOC, 1], t0_d)
        tlast_s = load_const("tlast_s", [BLOC, 1], tlast_d)

        nkap_s = const.tile([L, 1], F32, tag="nkap_s")
        nc.vector.memset(nkap_s[:], -KAPPA)
        expT_s = const.tile([L, L], BF16, tag="expT_s")
        nc.scalar.activation(expT_s[:], trans_s[:], AF.Exp, bias=nkap_s[:])
        sexp_s = const.tile([L, 1], F32, tag="sexp_s")
        nc.scalar.activation(sexp_s[:], startc_s[:], AF.Exp)
        eexp_s = const.tile([L, 1], BF16, tag="eexp_s")
        nc.scalar.activation(eexp_s[:], endc_s[:], AF.Exp)
        onesb_s = const.tile([L, 1], BF16, tag="onesb_s")
        nc.vector.memset(onesb_s[:], 1.0)
        identb_s = const.tile([L, L], BF16, tag="identb_s")
        nc.vector.tensor_copy(identb_s[:], ident_s[:])
        iotab_s = const.tile([L, L], BF16, tag="iotab_s")
        nc.vector.tensor_copy(iotab_s[:], iota_s[:])

        offset_s = offp.tile([1, BLOC], F32, tag="offset")
        nc.vector.memset(offset_s[:], 0.0)

        cmat = cp.tile([L, L], F32, tag="C")
        emitcol_s = const.tile([128, NCHUNK], F32, tag="emitcol")
        rbc = None   # pending renorm scale broadcast [L, BLOC]

        # Software pipelining by emission order: the Tile scheduler's
        # priority follows emission, and PE/DVE execute in-order, so each
        # helper op is emitted between scan steps where it fits inside that
        # step's engine-idle window instead of stalling the serial chain.
        def emit_load(cc):
            nat = natp.tile([128, L], F32, tag="nat")
            nc.sync.dma_start(nat[:], pred_d.ap()[bass.ts(cc, 128), :])
            natb = nbp.tile([128, L], BF16, tag="natb")
            nc.scalar.activation(natb[:], nat[:], AF.Copy)
            return nat, natb

        def emit_transpose(natb):
            pt = ptp.tile([L, 128], BF16, tag="pt")
            nc.tensor.transpose(pt[:], natb[:], identb_s[:])
            return pt

        def emit_exp(pt):
            P = pexp.tile([L, 128], F32, tag="P")
            nc.scalar.activation(P[:], pt[:], AF.Exp)
            return P

        # numerator for chunk pc, emitted piecewise (one DVE/PE insert per
        # scan step of the NEXT chunk so each fits that step's idle window)
        num_state = {}

        def emit_num_piece(pc, piece):
            if piece == 0:
                scr = scrp.tile([128, L], F32, tag="scr")
                nc.vector.scalar_tensor_tensor(
                    out=scr[:], in0=iota_s[:], scalar=tgtf_s[:, pc:pc + 1],
                    in1=num_state[pc]["nat"][:],
                    op0=OP.is_equal, op1=OP.mult,
                    accum_out=emitcol_s[:, pc:pc + 1])
            elif piece == 1:
                oh0 = ohp.tile([128, L], BF16, tag="oh0")
                nc.vector.tensor_scalar(
                    out=oh0[:], in0=iotab_s[:], scalar1=tgtf_s[:, pc:pc + 1],
                    scalar2=None, op0=OP.is_equal)
                num_state[pc]["oh0"] = oh0
            elif piece == 2:
                oh1 = ohp.tile([128, L], BF16, tag="oh1")
                nc.vector.tensor_scalar(
                    out=oh1[:], in0=iotab_s[:], scalar1=tgtn_s[:, pc:pc + 1],
                    scalar2=None, op0=OP.is_equal)
                num_state[pc]["oh1"] = oh1
            elif piece == 3:
                st = num_state.pop(pc)
                nc.tensor.matmul(cmat[:], st["oh0"][:], st["oh1"][:],
                                 start=(pc == 0), stop=(pc == NCHUNK - 1),
                                 skip_group_check=True)

        # prologue: chunk 0 fully prefetched
        nat_nxt, natb_nxt = emit_load(0)
        P_nxt = emit_exp(emit_transpose(natb_nxt))

        e = None
        for c in range(NCHUNK):
            nat_cur, natb_cur, P_cur = nat_nxt, natb_nxt, P_nxt
            num_state[c] = {"nat": nat_cur}

            # deferred renorm: fold pending 1/colsum into this chunk's first
            # P slice (reaches e via the next scan multiply; exact by
            # linearity)
            p0 = P_cur[:, 0:BLOC]
            if c % RENORM_EVERY == 0 and c > 0 and rbc is not None:
                psc = pscp.tile([L, BLOC], F32, tag="psc")
                nc.vector.tensor_tensor(out=psc[:], in0=P_cur[:, 0:BLOC],
                                        in1=rbc[:], op=OP.mult)
                p0 = psc[:]
                rbc = None

            for tl in range(TPC):
                t = c * TPC + tl
                pslice = p0 if tl == 0 else \
                    P_cur[:, tl * BLOC:(tl + 1) * BLOC]
                if t == 0:
                    e = ep.tile([L, BLOC], BF16, tag="e")
                    nc.vector.tensor_scalar(
                        out=e[:], in0=pslice, scalar1=sexp_s[:],
                        scalar2=None, op0=OP.mult)
                else:
                    z = zp.tile([L, BLOC], F32, tag="z")
                    nc.tensor.matmul(z[:], expT_s[:], e[:],
                                     start=True, stop=True)
                    e = ep.tile([L, BLOC], BF16, tag="e")
                    nc.vector.tensor_tensor(out=e[:], in0=z[:], in1=pslice,
                                            op=OP.mult)

                # off-chain renorm: colsum of e at t = 128k+120, k=0..6
                if t % (RENORM_EVERY * TPC) == 120 and t < (T - 128):
                    cs = rp.tile([1, BLOC], F32, tag="cs")
                    nc.tensor.matmul(cs[:], onesb_s[:], e[:],
                                     start=True, stop=True)
                    recip = smallp.tile([1, BLOC], F32, tag="recip")
                    nc.vector.reciprocal(recip[:], cs[:])
                    lnr = smallp.tile([1, BLOC], F32, tag="lnr")
                    nc.scalar.activation(lnr[:], recip[:], AF.Ln)
                    off_new = offp.tile([1, BLOC], F32, tag="offset")
                    nc.vector.tensor_tensor(
                        out=off_new[:], in0=offset_s[:], in1=lnr[:],
                        op=OP.subtract)
                    offset_s = off_new
                    rbc = rbcp.tile([L, BLOC], F32, tag="rbc")
                    nc.gpsimd.partition_broadcast(rbc[:], recip[:])

            # numerator work for this chunk - emitted AFTER the scan steps
            for piece in range(4):
                emit_num_piece(c, piece)

            # prefetch next chunk's P pipeline (emitted after this chunk's
            # scan ops -> lower priority, runs in this chunk's idle slots,
            # ready before the next chunk needs it)
            if c + 1 < NCHUNK:
                nat_nxt, natb_nxt = emit_load(c + 1)
                P_nxt = emit_exp(emit_transpose(natb_nxt))

        # ---- denominator finalization ----
        fz = rp.tile([1, BLOC], F32, tag="cs")
        nc.tensor.matmul(fz[:], eexp_s[:], e[:], start=True, stop=True)
        logden = smallp.tile([1, BLOC], F32, tag="logden")
        nc.scalar.activation(logden[:], fz[:], AF.Ln)
        den_row = smallp.tile([1, BLOC], F32, tag="denrow")
        nc.vector.tensor_tensor(out=den_row[:], in0=offset_s[:],
                                in1=logden[:], op=OP.add)
        den_tot = smallp.tile([1, 1], F32, tag="dentot")
        nc.vector.tensor_reduce(den_tot[:], den_row[:], AX.X, OP.add)

        # ---- numerator finalization ----
        emit_red = smallp.tile([128, 1], F32, tag="emitred")
        nc.vector.tensor_reduce(emit_red[:], emitcol_s[:], AX.X, OP.add)
        tscr = scrp.tile([L, L], F32, tag="scr")
        trans_red = smallp.tile([128, 1], F32, tag="transred")
        nc.vector.scalar_tensor_tensor(
            out=tscr[:], in0=cmat[:], scalar=1.0, in1=trans_s[:],
            op0=OP.mult, op1=OP.mult, accum_out=trans_red[:])
        num_col = smallp.tile([128, 1], F32, tag="numcol")
        nc.vector.tensor_tensor(out=num_col[:], in0=emit_red[:],
                                in1=trans_red[:], op=OP.add)
        num1 = rp.tile([1, 1], F32, tag="cs")
        nc.tensor.matmul(num1[:], num_col[:], ones_s[:], start=True, stop=True)

        # start/end gathers on 16 partitions
        sb16 = smallp.tile([BLOC, L], F32, tag="sb16")
        nc.gpsimd.partition_broadcast(sb16[:], startr_s[:])
        eb16 = smallp.tile([BLOC, L], F32, tag="eb16")
        nc.gpsimd.partition_broadcast(eb16[:], endr_s[:])
        s16 = smallp.tile([BLOC, L], F32, tag="s16scr")
        ssum = smallp.tile([BLOC, 1], F32, tag="ssum")
        nc.vector.scalar_tensor_tensor(
            out=s16[:], in0=iota_s[0:BLOC, :], scalar=t0_s[:], in1=sb16[:],
            op0=OP.is_equal, op1=OP.mult, accum_out=ssum[:])
        e16 = smallp.tile([BLOC, L], F32, tag="e16scr")
        esum = smallp.tile([BLOC, 1], F32, tag="esum")
        nc.vector.scalar_tensor_tensor(
            out=e16[:], in0=iota_s[0:BLOC, :], scalar=tlast_s[:], in1=eb16[:],
            op0=OP.is_equal, op1=OP.mult, accum_out=esum[:])
        se_col = smallp.tile([BLOC, 1], F32, tag="secol")
        nc.vector.tensor_tensor(out=se_col[:], in0=ssum[:], in1=esum[:],
                                op=OP.add)
        num2 = rp.tile([1, 1], F32, tag="cs")
        nc.tensor.matmul(num2[:], se_col[:], ones_s[0:BLOC, :],
                         start=True, stop=True)

        # partial = den_tot - num1 - num2
        p1 = smallp.tile([1, 1], F32, tag="p1")
        nc.vector.tensor_tensor(out=p1[:], in0=den_tot[:], in1=num1[:],
                                op=OP.subtract)
        p2 = smallp.tile([1, 1], F32, tag="p2")
        nc.vector.tensor_tensor(out=p2[:], in0=p1[:], in1=num2[:],
                                op=OP.subtract)
        nc.sync.dma_start(out_d.ap(), p2[:])

    nc.compile()
    return nc


_NC_CACHE = None


def _get_nc():
    global _NC_CACHE
    if _NC_CACHE is None:
        _NC_CACHE = _build_program()
    return _NC_CACHE


def _make_in_maps(predictions, targets, transitions, start_scores, end_scores):
    pred = np.ascontiguousarray(np.asarray(predictions, dtype=np.float32))
    tgt = np.asarray(targets).astype(np.int64)
    trans = np.ascontiguousarray(np.asarray(transitions, dtype=np.float32))
    start = np.asarray(start_scores, dtype=np.float32)
    end = np.asarray(end_scores, dtype=np.float32)

    iota = np.broadcast_to(np.arange(L, dtype=np.float32), (L, L)).copy()
    shared = {
        "transm": trans,
        "startc": start.reshape(L, 1).copy(),
        "endc": end.reshape(L, 1).copy(),
        "startr": start.reshape(1, L).copy(),
        "endr": end.reshape(1, L).copy(),
        "iotar": iota,
        "ident": np.eye(L, dtype=np.float32),
        "onesc": np.ones((L, 1), np.float32),
    }
    in_maps = []
    for core in range(NCORES):
        bsl = slice(core * BLOC, (core + 1) * BLOC)
        pred_c = np.ascontiguousarray(pred[:, bsl, :]).reshape(ROWS, L)
        tgt_c = tgt[:, bsl]                                   # [T, BLOC]
        tgtf = np.ascontiguousarray(
            tgt_c.astype(np.float32).reshape(NCHUNK, 128).T)  # [128, NCHUNK]
        tgtn_full = np.concatenate(
            [tgt_c[1:], np.full((1, BLOC), -1, np.int64)], axis=0)
        tgtn = np.ascontiguousarray(
            tgtn_full.astype(np.float32).reshape(NCHUNK, 128).T)
        in_maps.append({
            "pred": pred_c, "tgtf": tgtf, "tgtn": tgtn,
            "t0c": tgt_c[0].astype(np.float32).reshape(BLOC, 1).copy(),
            "tlastc": tgt_c[T - 1].astype(np.float32).reshape(BLOC, 1).copy(),
            **shared})
    return in_maps


def _finish(results):
    partials = [float(results[c]["out"].reshape(())) for c in range(NCORES)]
    return np.float32((sum(partials) + B * (T - 1) * KAPPA) / B)


def kernel(predictions, targets, mask, transitions, start_scores, end_scores):
    nc = _get_nc()
    in_maps = _make_in_maps(predictions, targets, transitions,
                            start_scores, end_scores)
    res = run_bass_kernel_spmd(nc, in_maps, list(range(NCORES)))
    return _finish(res.results)



# revision 4
# speedup vs baseline: 1.7750x; 1.7750x over previous
"""CRF loss (forward-algorithm log-partition minus gold-path score) on 8 TRN2
NeuronCores - bidirectional (forward+backward) scan.

Sharding: data-parallel over batch. B=128 -> 16 sequences per core; the small
(L,L) transition params are replicated.

The serial bottleneck of the forward algorithm is the per-step
matmul->multiply latency chain (~430ns/step on TRN2). This kernel halves the
chain length by scanning from BOTH ends simultaneously:

  fwd:  alpha_t = P_t (.) (expM^T alpha_{t-1}),  t = 1..512
  bwd:  c_t     = expM (P_{t+1} (.) c_{t+1}),    t = 1022..512
  Z_b  = sum_j alpha_512[j,b] * c_512[j,b]

with expM = exp(trans - kappa) in bf16 (stationary) and P = exp(pred) in
[label, (t,lane)] layout. The two chains are independent, so each engine
(PE matmul / DVE multiply) interleaves them and the wall time is one chain's
512-step latency instead of 1023 steps.

Layout: the host pre-transposes predictions into chunk-contiguous
[chunk][label][col] (col = 8 steps x 16 lanes) so chunks DMA as single 64KB
contiguous blocks straight into [128,128] SBUF tiles - no on-device
transpose. Exact per-lane renormalization every 128 steps per chain (colsum
measured 8 steps early, folded into a later P slice - off the critical path;
exact by linearity), with -ln(scale) tracked in offset rows.

Numerator: the emission sum (the only part that touches the 64MB pred
tensor) is computed on-device: the host sends a one-hot of the targets in
the same layout and each chunk contributes one fused multiply+reduce
(tensor_tensor_reduce) chained through a [128,1] accumulator. The
transition/start/end terms depend only on the small targets/params inputs
and are index arithmetic, done host-side along with the kappa offset and
the final mean (the scalar "all-reduce").
"""

import numpy as np
from contextlib import ExitStack

import concourse.bass as bass
import concourse.bacc as bacc
import concourse.tile as tile
from concourse import mybir
from concourse.bass_utils import run_bass_kernel_spmd

T, B, L = 1024, 128, 128
NCORES = 8
BLOC = B // NCORES          # 16 batch lanes per core
TPC = 8                     # time steps per 128-col chunk
NCHUNK = T // TPC           # 128 chunks
FSTEPS = T // 2             # fwd steps: t = 1..512
BSTEPS = T // 2 - 1         # bwd steps: k = 1..511 (t = 1023..513)
KAPPA = 5.9                 # mean per-step log growth; folded into expM
F32 = mybir.dt.float32
BF16 = mybir.dt.bfloat16
AX = mybir.AxisListType
OP = mybir.AluOpType
AF = mybir.ActivationFunctionType

RN_COLSUM = (120, 248, 376)   # measure 1/colsum at these steps (each chain)
RN_FOLD = (128, 256, 384)     # fold it into the P slice at these steps


def _build_program():
    nc = bacc.Bacc("TRN2", target_bir_lowering=False, debug=False,
                   num_devices=NCORES)

    pred_d = nc.dram_tensor("predc", [NCHUNK * 128, 128], F32,
                            kind="ExternalInput")
    oh_d = nc.dram_tensor("ohc", [NCHUNK * 128, 128], F32,
                          kind="ExternalInput")
    trans_d = nc.dram_tensor("transm", [L, L], F32, kind="ExternalInput")
    transt_d = nc.dram_tensor("transt", [L, L], F32, kind="ExternalInput")
    startc_d = nc.dram_tensor("startc", [L, 1], F32, kind="ExternalInput")
    endc_d = nc.dram_tensor("endc", [L, 1], F32, kind="ExternalInput")
    onesc_d = nc.dram_tensor("onesc", [L, 1], F32, kind="ExternalInput")
    den_d = nc.dram_tensor("outden", [1, BLOC], F32, kind="ExternalOutput")
    emit_d = nc.dram_tensor("outemit", [L, 1], F32, kind="ExternalOutput")

    with tile.TileContext(nc) as tc, ExitStack() as ctx:
        const = ctx.enter_context(tc.tile_pool(name="const", bufs=1))
        natfp = ctx.enter_context(tc.tile_pool(name="natf", bufs=3))
        pfp = ctx.enter_context(tc.tile_pool(name="pf", bufs=3))
        ohfp = ctx.enter_context(tc.tile_pool(name="ohf", bufs=3))
        natbp = ctx.enter_context(tc.tile_pool(name="natb", bufs=3))
        pbp = ctx.enter_context(tc.tile_pool(name="pb", bufs=3))
        ohbp = ctx.enter_context(tc.tile_pool(name="ohb", bufs=3))
        efp = ctx.enter_context(tc.tile_pool(name="ef", bufs=4))
        gbp = ctx.enter_context(tc.tile_pool(name="gb", bufs=4))
        scrp = ctx.enter_context(tc.tile_pool(name="scr", bufs=2))
        smallp = ctx.enter_context(tc.tile_pool(name="small", bufs=6))
        offp = ctx.enter_context(tc.tile_pool(name="off", bufs=4))
        rbcp = ctx.enter_context(tc.tile_pool(name="rbc", bufs=2))
        pscp = ctx.enter_context(tc.tile_pool(name="psc", bufs=2))
        zfp = ctx.enter_context(tc.tile_pool(name="zf", bufs=3, space="PSUM"))
        zbp = ctx.enter_context(tc.tile_pool(name="zb", bufs=3, space="PSUM"))
        rp = ctx.enter_context(tc.tile_pool(name="rsm", bufs=2, space="PSUM"))

        # ---- one-time constants ----
        def load_const(name, shape, dram):
            t = const.tile(shape, F32, tag=name)
            nc.sync.dma_start(t[:], dram.ap())
            return t

        trans_s = load_const("trans_s", [L, L], trans_d)
        transt_s = load_const("transt_s", [L, L], transt_d)
        startc_s = load_const("startc_s", [L, 1], startc_d)
        endc_s = load_const("endc_s", [L, 1], endc_d)
        onesc_s = load_const("onesc_s", [L, 1], onesc_d)

        nkap = const.tile([L, 1], F32, tag="nkap")
        nc.vector.memset(nkap[:], -KAPPA)
        expM = const.tile([L, L], BF16, tag="expM")
        nc.scalar.activation(expM[:], trans_s[:], AF.Exp, bias=nkap[:])
        expMT = const.tile([L, L], BF16, tag="expMT")
        nc.scalar.activation(expMT[:], transt_s[:], AF.Exp, bias=nkap[:])
        sexp = const.tile([L, 1], F32, tag="sexp")
        nc.scalar.activation(sexp[:], startc_s[:], AF.Exp)
        eexp = const.tile([L, 1], F32, tag="eexp")
        nc.scalar.activation(eexp[:], endc_s[:], AF.Exp)
        onesb = const.tile([L, 1], BF16, tag="onesb")
        nc.vector.memset(onesb[:], 1.0)
        ones16 = const.tile([L, BLOC], F32, tag="ones16")
        nc.vector.memset(ones16[:], 1.0)

        # ---- chunk pipelines ----
        fstate, bstate = {}, {}

        def load_chunk(c, natp, pp, ohp, store):
            nat = natp.tile([128, 128], F32, tag="nat")
            nc.sync.dma_start(nat[:], pred_d.ap()[bass.ts(c, 128), :])
            P = pp.tile([128, 128], F32, tag="P")
            nc.scalar.activation(P[:], nat[:], AF.Exp)
            oh = ohp.tile([128, 128], F32, tag="oh")
            nc.sync.dma_start(oh[:], oh_d.ap()[bass.ts(c, 128), :])
            store[c] = (nat, P, oh)

        def load_f(c):
            load_chunk(c, natfp, pfp, ohfp, fstate)

        def load_b(c):
            load_chunk(c, natbp, pbp, ohbp, bstate)

        # per-chunk emission accumulator columns (summed at the end)
        emitcol = const.tile([128, NCHUNK], F32, tag="emitcol")

        def emit_emission(nat, oh, c):
            scr = scrp.tile([128, 128], F32, tag="scr")
            nc.vector.scalar_tensor_tensor(
                out=scr[:], in0=oh[:], scalar=1.0, in1=nat[:],
                op0=OP.mult, op1=OP.mult,
                accum_out=emitcol[:, c:c + 1])

        # per-chain renorm state
        off = {}
        for w in ("f", "b"):
            t = offp.tile([1, BLOC], F32, tag=f"off{w}")
            nc.vector.memset(t[:], 0.0)
            off[w] = t
        pending = {"f": None, "b": None}

        def emit_colsum(state_bf16, w):
            cs = rp.tile([1, BLOC], F32, tag="cs")
            nc.tensor.matmul(cs[:], onesb[:], state_bf16[:],
                             start=True, stop=True)
            recip = smallp.tile([1, BLOC], F32, tag="recip")
            nc.vector.reciprocal(recip[:], cs[:])
            lnr = smallp.tile([1, BLOC], F32, tag="lnr")
            nc.scalar.activation(lnr[:], recip[:], AF.Ln)
            off_new = offp.tile([1, BLOC], F32, tag=f"off{w}")
            nc.vector.tensor_tensor(out=off_new[:], in0=off[w][:],
                                    in1=lnr[:], op=OP.subtract)
            off[w] = off_new
            rbc = rbcp.tile([L, BLOC], F32, tag="rbc")
            nc.gpsimd.partition_broadcast(rbc[:], recip[:])
            pending[w] = rbc

        def maybe_fold(pslice, w):
            if pending[w] is None:
                return pslice
            psc = pscp.tile([L, BLOC], F32, tag="psc")
            nc.vector.tensor_tensor(out=psc[:], in0=pslice, in1=pending[w][:],
                                    op=OP.mult)
            pending[w] = None
            return psc[:]

        # ---- prologue ----
        load_f(0)
        load_f(1)
        load_b(127)
        load_b(126)

        # alpha_0 = exp(start) (.) P_0   (t=0 -> chunk 0, cols 0..15)
        e_f = efp.tile([L, BLOC], BF16, tag="ef")
        nc.vector.tensor_scalar(out=e_f[:], in0=fstate[0][1][:, 0:BLOC],
                                scalar1=sexp[:], scalar2=None, op0=OP.mult)
        # c_1023 = exp(end), broadcast across lanes (f32 SBUF)
        cinit = smallp.tile([L, BLOC], F32, tag="cinit")
        nc.vector.tensor_scalar(out=cinit[:], in0=ones16[:],
                                scalar1=eexp[:], scalar2=None, op0=OP.mult)
        cur_cb = cinit[:]

        # ---- main bidirectional scan ----
        for r in range(1, FSTEPS + 1):
            # fwd matmul: zf = expM^T @ e_f
            zf = zfp.tile([L, BLOC], F32, tag="zf")
            nc.tensor.matmul(zf[:], expM[:], e_f[:], start=True, stop=True)

            # bwd multiply: g = P_{tb} (.) c  (tb = 1024-r)
            if r <= BSTEPS:
                tb = T - r
                cb_c, cb_tl = tb // TPC, tb % TPC
                pb = bstate[cb_c][1][:, cb_tl * BLOC:(cb_tl + 1) * BLOC]
                if r in RN_FOLD:
                    pb = maybe_fold(pb, "b")
                g = gbp.tile([L, BLOC], BF16, tag="g")
                nc.vector.tensor_tensor(out=g[:], in0=cur_cb, in1=pb,
                                        op=OP.mult)

            # fwd multiply: e_f = zf (.) P_r
            cf_c, cf_tl = r // TPC, r % TPC
            pf = fstate[cf_c][1][:, cf_tl * BLOC:(cf_tl + 1) * BLOC]
            if r in RN_FOLD:
                pf = maybe_fold(pf, "f")
            e_dt = F32 if r == FSTEPS else BF16
            e_f = efp.tile([L, BLOC], e_dt, tag="ef")
            nc.vector.tensor_tensor(out=e_f[:], in0=zf[:], in1=pf,
                                    op=OP.mult)

            # bwd matmul: c = expMT^T @ g  (= expM @ g)
            if r <= BSTEPS:
                zb = zbp.tile([L, BLOC], F32, tag="zb")
                nc.tensor.matmul(zb[:], expMT[:], g[:], start=True, stop=True)
                cur_cb = zb[:]

            # off-chain renorm bookkeeping
            if r in RN_COLSUM:
                emit_colsum(e_f, "f")
                emit_colsum(g, "b")

            # emission contributions, spread across the window
            if r % TPC == 3:
                m = r // TPC
                if m <= 63:
                    emit_emission(fstate[m][0], fstate[m][2], m)
            if r % TPC == 6:
                m = r // TPC
                emit_emission(bstate[127 - m][0], bstate[127 - m][2], 127 - m)

            # chunk prefetch at window boundaries
            if r % TPC == 0:
                m = r // TPC
                if m + 1 <= 64:
                    load_f(m + 1)
                if m <= 62:
                    load_b(126 - m)
                # drop stale chunk refs so pools can recycle
                fstate.pop(m - 1, None)
                bstate.pop(128 - m, None)

        # ---- finalization ----
        u = smallp.tile([L, BLOC], F32, tag="u")
        nc.vector.tensor_tensor(out=u[:], in0=cur_cb, in1=e_f[:], op=OP.mult)
        fz = rp.tile([1, BLOC], F32, tag="cs")
        nc.tensor.matmul(fz[:], onesc_s[:], u[:], start=True, stop=True)
        lnz = smallp.tile([1, BLOC], F32, tag="lnz")
        nc.scalar.activation(lnz[:], fz[:], AF.Ln)
        d1 = smallp.tile([1, BLOC], F32, tag="d1")
        nc.vector.tensor_tensor(out=d1[:], in0=lnz[:], in1=off["f"][:],
                                op=OP.add)
        d2 = smallp.tile([1, BLOC], F32, tag="d2")
        nc.vector.tensor_tensor(out=d2[:], in0=d1[:], in1=off["b"][:],
                                op=OP.add)
        nc.sync.dma_start(den_d.ap(), d2[:])
        emitred = smallp.tile([128, 1], F32, tag="emitred")
        nc.vector.tensor_reduce(emitred[:], emitcol[:], AX.X, OP.add)
        nc.sync.dma_start(emit_d.ap(), emitred[:])

    nc.compile()
    return nc


_NC_CACHE = None


def _get_nc():
    global _NC_CACHE
    if _NC_CACHE is None:
        _NC_CACHE = _build_program()
    return _NC_CACHE


_HOST_NUM = {"v": 0.0}


def _make_in_maps(predictions, targets, transitions, start_scores, end_scores):
    pred = np.ascontiguousarray(np.asarray(predictions, dtype=np.float32))
    tgt = np.asarray(targets).astype(np.int64)
    trans = np.ascontiguousarray(np.asarray(transitions, dtype=np.float32))
    start = np.asarray(start_scores, dtype=np.float32)
    end = np.asarray(end_scores, dtype=np.float32)

    # host-side numerator pieces that touch only targets + small params
    # (mask is all ones in this benchmark, as the baseline also assumes)
    tr_sum = float(trans[tgt[:-1], tgt[1:]].sum(dtype=np.float64))
    se_sum = float(start[tgt[0]].sum(dtype=np.float64)
                   + end[tgt[-1]].sum(dtype=np.float64))
    _HOST_NUM["v"] = tr_sum + se_sum

    shared = {
        "transm": trans,
        "transt": np.ascontiguousarray(trans.T),
        "startc": start.reshape(L, 1).copy(),
        "endc": end.reshape(L, 1).copy(),
        "onesc": np.ones((L, 1), np.float32),
    }
    iota = np.arange(L, dtype=np.int64)
    in_maps = []
    for core in range(NCORES):
        bsl = slice(core * BLOC, (core + 1) * BLOC)
        # [T, BLOC, L] -> [chunk, L, col] with col = tl*BLOC + lane
        pc = pred[:, bsl, :].reshape(NCHUNK, TPC, BLOC, L)
        predc = np.ascontiguousarray(
            pc.transpose(0, 3, 1, 2)).reshape(NCHUNK * 128, 128)
        tcol = tgt[:, bsl].reshape(NCHUNK, TPC * BLOC)   # [chunk, col]
        ohc = (tcol[:, None, :] == iota[None, :, None]).astype(np.float32)
        in_maps.append({
            "predc": predc,
            "ohc": np.ascontiguousarray(ohc).reshape(NCHUNK * 128, 128),
            **shared})
    return in_maps


def _finish(results):
    den = 0.0
    emit = 0.0
    for c in range(NCORES):
        den += float(results[c]["outden"].astype(np.float64).sum())
        emit += float(results[c]["outemit"].astype(np.float64).sum())
    den += B * (T - 1) * KAPPA
    return np.float32((den - emit - _HOST_NUM["v"]) / B)


def kernel(predictions, targets, mask, transitions, start_scores, end_scores):
    nc = _get_nc()
    in_maps = _make_in_maps(predictions, targets, transitions,
                            start_scores, end_scores)
    res = run_bass_kernel_spmd(nc, in_maps, list(range(NCORES)))
    return _finish(res.results)


# revision 5
# speedup vs baseline: 1.8929x; 1.0664x over previous
"""CRF loss (forward-algorithm log-partition minus gold-path score) on 8 TRN2
NeuronCores - bidirectional (forward+backward) scan.

Sharding: data-parallel over batch. B=128 -> 16 sequences per core; the small
(L,L) transition params are replicated.

The serial bottleneck of the forward algorithm is the per-step
matmul->multiply latency chain (~430ns/step on TRN2). This kernel halves the
chain length by scanning from BOTH ends simultaneously:

  fwd:  alpha_t = P_t (.) (expM^T alpha_{t-1}),  t = 1..512
  bwd:  c_t     = expM (P_{t+1} (.) c_{t+1}),    t = 1022..512
  Z_b  = sum_j alpha_512[j,b] * c_512[j,b]

with expM = exp(trans - kappa) in bf16 (stationary) and P = exp(pred) in
[label, (t,lane)] layout. The two chains are independent, so each engine
(PE matmul / DVE multiply) interleaves them and the wall time is one chain's
512-step latency instead of 1023 steps.

Layout: the host pre-transposes predictions into chunk-contiguous
[chunk][label][col] (col = 8 steps x 16 lanes); chunk PAIRS stream as single
128KB contiguous DMAs straight into [128,256] SBUF tiles - no on-device
transpose. Exact per-lane renormalization every 128 steps per chain (colsum
measured 8 steps early, folded into a later P slice - off the critical path;
exact by linearity). The raw colsums and the final Z row are exported and
the host takes the logs - this keeps the Scalar engine's activation table
pinned to EXP (a device-side Ln costs ~2.6us per table swap).

Numerator: the emission sum (the only part that touches the 64MB pred
tensor) is computed on-device: the host sends a one-hot of the targets in
the same layout and each chunk pair contributes one fused
scalar_tensor_tensor multiply+accumulate on the idle slots of the Vector
engine. The transition/start/end terms depend only on the small
targets/params inputs and are index arithmetic, done host-side along with
the kappa offset, the logs, and the final mean (the scalar "all-reduce").
"""

import numpy as np
from contextlib import ExitStack

import concourse.bass as bass
import concourse.bacc as bacc
import concourse.tile as tile
from concourse import mybir
from concourse.bass_utils import run_bass_kernel_spmd

T, B, L = 1024, 128, 128
NCORES = 8
BLOC = B // NCORES          # 16 batch lanes per core
TPC = 8                     # time steps per 128-col chunk
NCHUNK = T // TPC           # 128 chunks
TPP = 16                    # time steps per chunk pair
NPAIR = T // TPP            # 64 chunk pairs
FSTEPS = T // 2             # fwd steps: t = 1..512
BSTEPS = T // 2 - 1         # bwd steps: k = 1..511 (t = 1023..513)
KAPPA = 5.9                 # mean per-step log growth; folded into expM
F32 = mybir.dt.float32
BF16 = mybir.dt.bfloat16
AX = mybir.AxisListType
OP = mybir.AluOpType
AF = mybir.ActivationFunctionType

RN_COLSUM = (120, 248, 376)   # measure colsums at these steps (each chain)
RN_FOLD = (128, 256, 384)     # fold 1/colsum into the P slice at these steps
NCS = 2 * len(RN_COLSUM) + 1  # exported rows: 6 colsums + final Z


def _build_program():
    nc = bacc.Bacc("TRN2", target_bir_lowering=False, debug=False,
                   num_devices=NCORES)

    pred_d = nc.dram_tensor("predc", [NCHUNK * 128, 128], F32,
                            kind="ExternalInput")
    oh_d = nc.dram_tensor("ohc", [NCHUNK * 128, 128], F32,
                          kind="ExternalInput")
    trans_d = nc.dram_tensor("transm", [L, L], F32, kind="ExternalInput")
    transt_d = nc.dram_tensor("transt", [L, L], F32, kind="ExternalInput")
    startc_d = nc.dram_tensor("startc", [L, 1], F32, kind="ExternalInput")
    endc_d = nc.dram_tensor("endc", [L, 1], F32, kind="ExternalInput")
    cs_d = nc.dram_tensor("outcs", [1, NCS * BLOC], F32,
                          kind="ExternalOutput")
    emit_d = nc.dram_tensor("outemit", [L, 1], F32, kind="ExternalOutput")

    with tile.TileContext(nc) as tc, ExitStack() as ctx:
        const = ctx.enter_context(tc.tile_pool(name="const", bufs=1))
        natfp = ctx.enter_context(tc.tile_pool(name="natf", bufs=3))
        pfp = ctx.enter_context(tc.tile_pool(name="pf", bufs=3))
        ohfp = ctx.enter_context(tc.tile_pool(name="ohf", bufs=3))
        natbp = ctx.enter_context(tc.tile_pool(name="natb", bufs=3))
        pbp = ctx.enter_context(tc.tile_pool(name="pb", bufs=3))
        ohbp = ctx.enter_context(tc.tile_pool(name="ohb", bufs=3))
        efp = ctx.enter_context(tc.tile_pool(name="ef", bufs=6))
        gbp = ctx.enter_context(tc.tile_pool(name="gb", bufs=6))
        scrp = ctx.enter_context(tc.tile_pool(name="scr", bufs=2))
        smallp = ctx.enter_context(tc.tile_pool(name="small", bufs=6))
        rbcp = ctx.enter_context(tc.tile_pool(name="rbc", bufs=2))
        pscp = ctx.enter_context(tc.tile_pool(name="psc", bufs=2))
        zfp = ctx.enter_context(tc.tile_pool(name="zf", bufs=3, space="PSUM"))
        zbp = ctx.enter_context(tc.tile_pool(name="zb", bufs=3, space="PSUM"))
        rp = ctx.enter_context(tc.tile_pool(name="rsm", bufs=2, space="PSUM"))

        # ---- one-time constants ----
        def load_const(name, shape, dram):
            t = const.tile(shape, F32, tag=name)
            nc.sync.dma_start(t[:], dram.ap())
            return t

        trans_s = load_const("trans_s", [L, L], trans_d)
        transt_s = load_const("transt_s", [L, L], transt_d)
        startc_s = load_const("startc_s", [L, 1], startc_d)
        endc_s = load_const("endc_s", [L, 1], endc_d)

        nkap = const.tile([L, 1], F32, tag="nkap")
        nc.vector.memset(nkap[:], -KAPPA)
        expM = const.tile([L, L], BF16, tag="expM")
        nc.scalar.activation(expM[:], trans_s[:], AF.Exp, bias=nkap[:])
        expMT = const.tile([L, L], BF16, tag="expMT")
        nc.scalar.activation(expMT[:], transt_s[:], AF.Exp, bias=nkap[:])
        sexp = const.tile([L, 1], F32, tag="sexp")
        nc.scalar.activation(sexp[:], startc_s[:], AF.Exp)
        eexp = const.tile([L, 1], F32, tag="eexp")
        nc.scalar.activation(eexp[:], endc_s[:], AF.Exp)
        onesb = const.tile([L, 1], BF16, tag="onesb")
        nc.vector.memset(onesb[:], 1.0)
        ones16 = const.tile([L, BLOC], F32, tag="ones16")
        nc.vector.memset(ones16[:], 1.0)
        onesf = const.tile([L, 1], F32, tag="onesf")
        nc.vector.memset(onesf[:], 1.0)

        # exported colsum/Z rows and per-chunk emission accumulators
        csout = const.tile([1, NCS * BLOC], F32, tag="csout")
        emitcol = const.tile([128, NPAIR], F32, tag="emitcol")

        # ---- chunk-pair pipelines ----
        fstate, bstate = {}, {}

        def load_pair(p, natp, pp, ohp, store):
            nat = natp.tile([128, 256], F32, tag="nat")
            nc.sync.dma_start(nat[:], pred_d.ap()[bass.ts(p, 256), :])
            P = pp.tile([128, 256], F32, tag="P")
            nc.scalar.activation(P[:], nat[:], AF.Exp)
            oh = ohp.tile([128, 256], F32, tag="oh")
            nc.sync.dma_start(oh[:], oh_d.ap()[bass.ts(p, 256), :])
            store[p] = (nat, P, oh)

        def load_f(p):
            load_pair(p, natfp, pfp, ohfp, fstate)

        def load_b(p):
            load_pair(p, natbp, pbp, ohbp, bstate)

        def emit_emission(pair, store):
            nat, _, oh = store[pair]
            scr = scrp.tile([128, 256], F32, tag="scr")
            nc.vector.scalar_tensor_tensor(
                out=scr[:], in0=oh[:], scalar=1.0, in1=nat[:],
                op0=OP.mult, op1=OP.mult,
                accum_out=emitcol[:, pair:pair + 1])

        # per-chain renorm state
        pending = {"f": None, "b": None}
        ncs_used = [0]

        def emit_colsum(state_bf16, w):
            cs = rp.tile([1, BLOC], F32, tag="cs")
            nc.tensor.matmul(cs[:], onesb[:], state_bf16[:],
                             start=True, stop=True)
            i = ncs_used[0]
            ncs_used[0] += 1
            nc.vector.tensor_copy(csout[:, i * BLOC:(i + 1) * BLOC], cs[:])
            recip = smallp.tile([1, BLOC], F32, tag="recip")
            nc.vector.reciprocal(recip[:], cs[:])
            rbc = rbcp.tile([L, BLOC], F32, tag="rbc")
            nc.gpsimd.partition_broadcast(rbc[:], recip[:])
            pending[w] = rbc

        def maybe_fold(pslice, w):
            if pending[w] is None:
                return pslice
            psc = pscp.tile([L, BLOC], F32, tag="psc")
            nc.vector.tensor_tensor(out=psc[:], in0=pslice, in1=pending[w][:],
                                    op=OP.mult)
            pending[w] = None
            return psc[:]

        # ---- prologue ----
        load_f(0)
        load_f(1)
        load_b(NPAIR - 1)
        load_b(NPAIR - 2)

        # alpha_0 = exp(start) (.) P_0   (t=0 -> pair 0, cols 0..15)
        e_f = efp.tile([L, BLOC], BF16, tag="ef")
        nc.vector.tensor_scalar(out=e_f[:], in0=fstate[0][1][:, 0:BLOC],
                                scalar1=sexp[:], scalar2=None, op0=OP.mult)
        # c_1023 = exp(end), broadcast across lanes (f32 SBUF)
        cinit = smallp.tile([L, BLOC], F32, tag="cinit")
        nc.vector.tensor_scalar(out=cinit[:], in0=ones16[:],
                                scalar1=eexp[:], scalar2=None, op0=OP.mult)
        cur_cb = cinit[:]

        # ---- main bidirectional scan ----
        for r in range(1, FSTEPS + 1):
            # fwd matmul: zf = expM^T @ e_f
            zf = zfp.tile([L, BLOC], F32, tag="zf")
            nc.tensor.matmul(zf[:], expM[:], e_f[:], start=True, stop=True)

            # bwd multiply: g = P_{tb} (.) c  (tb = 1024-r)
            if r <= BSTEPS:
                tb = T - r
                bp, btl = tb // TPP, tb % TPP
                pb = bstate[bp][1][:, btl * BLOC:(btl + 1) * BLOC]
                if r in RN_FOLD:
                    pb = maybe_fold(pb, "b")
                g = gbp.tile([L, BLOC], BF16, tag="g")
                nc.vector.tensor_tensor(out=g[:], in0=cur_cb, in1=pb,
                                        op=OP.mult)

            # fwd multiply: e_f = zf (.) P_r
            fp_, ftl = r // TPP, r % TPP
            pf = fstate[fp_][1][:, ftl * BLOC:(ftl + 1) * BLOC]
            if r in RN_FOLD:
                pf = maybe_fold(pf, "f")
            e_dt = F32 if r == FSTEPS else BF16
            e_f = efp.tile([L, BLOC], e_dt, tag="ef")
            nc.vector.tensor_tensor(out=e_f[:], in0=zf[:], in1=pf,
                                    op=OP.mult)

            # bwd matmul: c = expM @ g
            if r <= BSTEPS:
                zb = zbp.tile([L, BLOC], F32, tag="zb")
                nc.tensor.matmul(zb[:], expMT[:], g[:], start=True, stop=True)
                cur_cb = zb[:]

            # off-chain renorm bookkeeping (logs taken on the host)
            if r in RN_COLSUM:
                emit_colsum(e_f, "f")
                emit_colsum(g, "b")

            # emission contributions, spread across the window
            if r % TPP == 3:
                m = r // TPP
                if m <= 31:
                    emit_emission(m, fstate)
            if r % TPP == 11:
                m = r // TPP
                emit_emission(NPAIR - 1 - m, bstate)

            # pair prefetch at window boundaries
            if r % TPP == 0:
                m = r // TPP
                if m + 1 <= NPAIR // 2:
                    load_f(m + 1)
                if m <= 30:
                    load_b(NPAIR - 2 - m)
                fstate.pop(m - 1, None)
                bstate.pop(NPAIR - m, None)

        # ---- finalization: Z row exported, host takes the log ----
        u = smallp.tile([L, BLOC], F32, tag="u")
        nc.vector.tensor_tensor(out=u[:], in0=cur_cb, in1=e_f[:], op=OP.mult)
        fz = rp.tile([1, BLOC], F32, tag="cs")
        nc.tensor.matmul(fz[:], onesf[:], u[:], start=True, stop=True)
        nc.vector.tensor_copy(csout[:, NCS * BLOC - BLOC:], fz[:])
        nc.sync.dma_start(cs_d.ap(), csout[:])
        emitred = smallp.tile([128, 1], F32, tag="emitred")
        nc.vector.tensor_reduce(emitred[:], emitcol[:], AX.X, OP.add)
        nc.sync.dma_start(emit_d.ap(), emitred[:])

    nc.compile()
    return nc


_NC_CACHE = None


def _get_nc():
    global _NC_CACHE
    if _NC_CACHE is None:
        _NC_CACHE = _build_program()
    return _NC_CACHE


_HOST_NUM = {"v": 0.0}


def _make_in_maps(predictions, targets, transitions, start_scores, end_scores):
    pred = np.ascontiguousarray(np.asarray(predictions, dtype=np.float32))
    tgt = np.asarray(targets).astype(np.int64)
    trans = np.ascontiguousarray(np.asarray(transitions, dtype=np.float32))
    start = np.asarray(start_scores, dtype=np.float32)
    end = np.asarray(end_scores, dtype=np.float32)

    # host-side numerator pieces that touch only targets + small params
    # (mask is all ones in this benchmark, as the baseline also assumes)
    tr_sum = float(trans[tgt[:-1], tgt[1:]].sum(dtype=np.float64))
    se_sum = float(start[tgt[0]].sum(dtype=np.float64)
                   + end[tgt[-1]].sum(dtype=np.float64))
    _HOST_NUM["v"] = tr_sum + se_sum

    shared = {
        "transm": trans,
        "transt": np.ascontiguousarray(trans.T),
        "startc": start.reshape(L, 1).copy(),
        "endc": end.reshape(L, 1).copy(),
    }
    iota = np.arange(L, dtype=np.int64)
    in_maps = []
    for core in range(NCORES):
        bsl = slice(core * BLOC, (core + 1) * BLOC)
        # [T, BLOC, L] -> [chunk, L, col] with col = tl*BLOC + lane
        pc = pred[:, bsl, :].reshape(NCHUNK, TPC, BLOC, L)
        predc = np.ascontiguousarray(
            pc.transpose(0, 3, 1, 2)).reshape(NCHUNK * 128, 128)
        tcol = tgt[:, bsl].reshape(NCHUNK, TPC * BLOC)   # [chunk, col]
        ohc = (tcol[:, None, :] == iota[None, :, None]).astype(np.float32)
        in_maps.append({
            "predc": predc,
            "ohc": np.ascontiguousarray(ohc).reshape(NCHUNK * 128, 128),
            **shared})
    return in_maps


def _finish(results):
    den = 0.0
    emit = 0.0
    for c in range(NCORES):
        cs = results[c]["outcs"].astype(np.float64).reshape(NCS, BLOC)
        den += float(np.log(cs).sum())
        emit += float(results[c]["outemit"].astype(np.float64).sum())
    den += B * (T - 1) * KAPPA
    return np.float32((den - emit - _HOST_NUM["v"]) / B)


def kernel(predictions, targets, mask, transitions, start_scores, end_scores):
    nc = _get_nc()
    in_maps = _make_in_maps(predictions, targets, transitions,
                            start_scores, end_scores)
    res = run_bass_kernel_spmd(nc, in_maps, list(range(NCORES)))
    return _finish(res.results)
